# revision 1
# baseline (speedup 1.0000x reference)
"""Trainium2 Bass kernel for nn_DeltaModel (histogram_binning).

Reference semantics (delta == 0, the shipped configuration):
  med[t,ch]   = lower median over N of logits[t,:,ch]          (rows 0-4 used)
  q[n,ch]     = sumsq - 0.1*sum^2  over the 10 rows            (9*var*... monotone in std)
  std_med[ch] = sqrt(median_N(q[:,ch]) / 9)
  mode[n,ch]  = (#{t<5: logits[t,n,ch] >= med[t,ch] + 1.96*std_med[ch]} >= 3)
  c           = broadcast(mode) over dim 0
  out[t,:,ch] = xs[t,ch] - logsumexp(xs[t,others(ch)])  (constant over N)

Device work is split into three SPMD launches over 8 NeuronCores:
  L1 "stats+med": per-core column shard -> q shard; plus 3 assigned full
      (t,ch) slices -> exact-to-3e-8 medians via bisection counting.
  L2 "qmed": per-core one channel of the assembled q array -> its median.
  L3 "mode": per-core column shard rows 0-4 + thresholds -> mode shard.
Host does only sharding/padding, tiny scalar combination of the reduction
results, and broadcast-view assembly of the two full outputs.
"""

import numpy as np

LAST_RUN_TIMES = []  # wall seconds of each device launch (incl. first-call compile)

N = 1_000_000
NCORES = 8
SHARD = N // NCORES            # 125000
PADW_PP = 992                  # per-partition padded columns (16 x 62)
SHARD_PAD = 128 * PADW_PP      # 126976
SLICE_FREE = 7816              # per-partition elements of a 1M slice
SLICE_PAD = 128 * SLICE_FREE   # 1000448
PAD_BIG = np.float32(1e30)
LEVELS_MED = 16
LEVELS_Q = 12
RANK = 500000.0
FACTOR = np.float32(1.96)
# Brackets are ~15+ sigma certain for iid N(0,1) inputs; the host re-derives
# any median whose bisection lands on a bracket boundary (never in practice).
MED_RANGE = (-0.02, 0.02)
Q_RANGE = (8.2, 8.5)


def _apply_tile_patch():
    """This walrus build rejects >2 sync waits on the SP Drain emitted at
    TileContext exit ("Too many sync wait commands"); keep one wait on the
    drain and move the rest onto dedicated SP nops before the barrier."""
    import concourse.tile as tile_mod
    from concourse import mybir
    from concourse.vector_clock import ScopedClock

    if getattr(tile_mod.TileContext, "_ant_drain_patched", False):
        return

    def _patched(self, tick_clock, wait_clock):
        nc = self.nc
        drain_inst = nc.sync.drain()
        wait_clock.add_sem_waits(
            drain_inst.ins, ScopedClock({None: tick_clock.global_clock})
        )
        si = drain_inst.ins.sync_info
        if si is not None and si.on_wait is not None and len(si.on_wait) > 1:
            waits = list(si.on_wait)
            drain_inst.ins.sync_info = mybir.SyncInfo(
                on_wait=waits[:1], on_update=list(si.on_update or [])
            )
            for w in waits[1:]:
                nop = nc.sync.nop()
                nop.ins.sync_info = mybir.SyncInfo(on_wait=[w], on_update=[])
        nc.all_engine_barrier()
        assert self.sems is not None
        popped = nc._tile_sem_poison_stack.pop()
        assert popped is self._sem_poison
        nc.clear_and_free_semaphores(list(self.sems.allocated().values()))
        nc.all_engine_barrier()

    tile_mod.TileContext._drain_and_barrier = _patched
    tile_mod.TileContext._ant_drain_patched = True


def _split_sync_waits(nc, maxw=1):
    """This walrus build caps per-instruction sync waits; move excess waits
    onto same-engine NoOps inserted right before the offending instruction."""
    from concourse import mybir

    for f in nc.m.functions:
        for b in f.blocks:
            new_list = []
            changed = False
            for ins in b.instructions:
                si = getattr(ins, "sync_info", None)
                if si is not None and si.on_wait and len(si.on_wait) > maxw:
                    waits = list(si.on_wait)
                    extra, keep = waits[:-maxw], waits[-maxw:]
                    for i in range(0, len(extra), maxw):
                        nop = mybir.InstNoOp(
                            name=f"{ins.name}-wsplit{i}", ins=[], outs=[]
                        )
                        nop.engine = ins.engine
                        nop.sync_info = mybir.SyncInfo(
                            on_wait=extra[i:i + maxw], on_update=[]
                        )
                        new_list.append(nop)
                        changed = True
                    ins.sync_info = mybir.SyncInfo(
                        on_wait=keep, on_update=list(si.on_update or [])
                    )
                new_list.append(ins)
            if changed:
                b.instructions = new_list


def _bisect_median(nc, pool, psum, ones, data_tiles, state_tiles, junk, levels,
                   rank, n_padded, act_slices=(), sgn_junk=None):
    """Shared bisection loop: for each slice s, refine [lo, lo+2h) containing
    the rank-`rank` smallest element of data_tiles[s] (n_padded elements with
    pads at +1e30).  state cols: 0=lo 1=h 2=mid 3=acc 4=nmid (all [128,1]).
    Slices in act_slices count on the Scalar engine via sign-sums (ties count
    half, shifting the located interval by at most one float step - harmless
    at our tolerance); the rest count on the Vector engine."""
    from concourse import mybir

    S = len(data_tiles)
    maskt = pool.tile([128, S], mybir.dt.int32, name="maskt")
    # count(x < mid) < rank  <=>  sum(sign(x - mid)) > n_padded - 2*rank
    sgn_thresh = float(n_padded - 2 * rank)
    for _ in range(levels):
        for s in range(S):
            st = state_tiles[s]
            lo, h, mid = st[:, 0:1], st[:, 1:2], st[:, 2:3]
            acc, cmp = st[:, 3:4], maskt[:, s:s + 1]
            nc.vector.tensor_tensor(out=mid, in0=lo, in1=h, op=mybir.AluOpType.add)
            tot = psum.tile([128, 1], mybir.dt.float32, tag=f"tot{s}", name=f"tot{s}")
            if s in act_slices:
                nmid = st[:, 4:5]
                nc.vector.scalar_tensor_tensor(
                    out=nmid, in0=lo, scalar=-1.0, in1=h,
                    op0=mybir.AluOpType.mult, op1=mybir.AluOpType.subtract,
                )
                nc.scalar.activation(
                    out=sgn_junk, in_=data_tiles[s],
                    func=mybir.ActivationFunctionType.Sign,
                    bias=nmid, scale=1.0, accum_out=acc,
                )
                nc.tensor.matmul(tot, lhsT=ones, rhs=acc, start=True, stop=True)
                nc.vector.tensor_scalar(
                    out=cmp, in0=tot, scalar1=sgn_thresh, scalar2=None,
                    op0=mybir.AluOpType.is_gt,
                )
            else:
                nc.vector.tensor_scalar(
                    out=junk, in0=data_tiles[s], scalar1=mid, scalar2=None,
                    op0=mybir.AluOpType.is_lt, op1=mybir.AluOpType.add,
                    accum_out=acc,
                )
                nc.tensor.matmul(tot, lhsT=ones, rhs=acc, start=True, stop=True)
                nc.vector.tensor_scalar(
                    out=cmp, in0=tot, scalar1=rank, scalar2=None,
                    op0=mybir.AluOpType.is_lt,
                )
            # where the median is above mid: lo <- mid
            nc.vector.copy_predicated(out=lo, mask=cmp, data=mid)
            nc.vector.tensor_scalar(
                out=h, in0=h, scalar1=0.5, scalar2=None, op0=mybir.AluOpType.mult
            )


def build_l1(nslices=3, slice_free=SLICE_FREE, padw_pp=PADW_PP, nrows=10,
             levels=LEVELS_MED, rank=RANK, use_act=True, pe_stats=True,
             split_waits=True):
    """L1: column-shard stats (q = sumsq - 0.1*sum^2) + bisection medians of
    `nslices` full slices."""
    import concourse.bass as bass
    import concourse.tile as tile
    from concourse import mybir

    _apply_tile_patch()
    chunk_pp = padw_pp // 16
    qw = padw_pp * 4
    nc = bass.Bass("TRN2", target_bir_lowering=False, debug=False, num_devices=1)
    shard = nc.dram_tensor("shardpad", [nrows, 128 * padw_pp, 4], mybir.dt.float32,
                           kind="ExternalInput").ap()
    slices = nc.dram_tensor("slices", [nslices, 128 * slice_free], mybir.dt.float32,
                            kind="ExternalInput").ap()
    ranges = nc.dram_tensor("ranges", [nslices, 2], mybir.dt.float32,
                            kind="ExternalInput").ap()
    identd = nc.dram_tensor("ident", [128, 128], mybir.dt.float32,
                            kind="ExternalInput").ap()
    qvar = nc.dram_tensor("qvar", [128, qw], mybir.dt.float32,
                          kind="ExternalOutput").ap()
    med = nc.dram_tensor("med", [1, nslices], mybir.dt.float32,
                         kind="ExternalOutput").ap()

    with tile.TileContext(nc) as tc:
        with tc.tile_pool(name="sl", bufs=1) as slpool, \
             tc.tile_pool(name="stream", bufs=2) as stream, \
             tc.tile_pool(name="scr", bufs=1) as scr, \
             tc.tile_pool(name="stat", bufs=1) as stat, \
             tc.tile_pool(name="small", bufs=1) as small, \
             tc.tile_pool(name="ps", bufs=1, space="PSUM") as psum, \
             tc.tile_pool(name="pstat", bufs=2, space="PSUM") as pstat:
            ones = small.tile([128, 128], mybir.dt.float32)
            nc.vector.memset(ones, 1.0)
            ident = small.tile([128, 128], mybir.dt.float32)
            nc.sync.dma_start(out=ident, in_=identd)
            junk = small.tile([128, slice_free], mybir.dt.bfloat16, name="junk")
            sgnj = small.tile([128, slice_free], mybir.dt.bfloat16, name="sgnj")

            state_all = small.tile([128, 8 * nslices], mybir.dt.float32)
            data_tiles, state_tiles = [], []
            for s in range(nslices):
                d = slpool.tile([128, slice_free], mybir.dt.float32, tag=f"d{s}", name=f"d{s}")
                nc.sync.dma_start(
                    out=d, in_=slices[s].rearrange("(p f) -> p f", p=128)
                )
                st = state_all[:, 8 * s:8 * s + 8]
                nc.sync.dma_start(
                    out=st[:, 0:2],
                    in_=bass.AP(tensor=ranges.tensor, offset=s * 2,
                                ap=[[0, 128], [1, 2]]),
                )
                data_tiles.append(d)
                state_tiles.append(st)

            _bisect_median(nc, small, psum, ones, data_tiles, state_tiles,
                           junk, levels, rank, n_padded=128 * slice_free,
                           act_slices=(2,) if (use_act and nslices > 2) else (),
                           sgn_junk=sgnj)

            medt = small.tile([1, nslices], mybir.dt.float32)
            for s in range(nslices):
                st = state_tiles[s]
                nc.vector.tensor_tensor(out=medt[:, s:s + 1], in0=st[0:1, 0:1],
                                        in1=st[0:1, 1:2], op=mybir.AluOpType.add)
            nc.sync.dma_start(out=med, in_=medt)

            # ---- stats over the 10 rows ----
            free = chunk_pp * 4
            for it in range(16):
                ld = stream.tile([128, nrows, free], mybir.dt.float32, tag="ld")
                src = bass.AP(
                    tensor=shard.tensor,
                    offset=it * chunk_pp * 4,
                    ap=[[padw_pp * 4, 128], [128 * padw_pp * 4, nrows],
                        [4, chunk_pp], [1, 4]],
                )
                nc.sync.dma_start(out=ld.rearrange("p t (c k) -> p t c k", k=4), in_=src)
                sq = scr.tile([128, nrows, free], mybir.dt.float32, tag="scr",
                              name="sq")
                nc.scalar.activation(out=sq, in_=ld,
                                     func=mybir.ActivationFunctionType.Square)
                if pe_stats:
                    sum_acc = pstat.tile([128, free], mybir.dt.float32, tag="sum",
                                         name="sum_ps")
                    ssq_acc = pstat.tile([128, free], mybir.dt.float32, tag="ssq",
                                         name="ssq_ps")
                    for t in range(nrows):
                        nc.tensor.matmul(sum_acc, lhsT=ident, rhs=ld[:, t, :],
                                         start=(t == 0), stop=(t == nrows - 1))
                    for t in range(nrows):
                        nc.tensor.matmul(ssq_acc, lhsT=ident, rhs=sq[:, t, :],
                                         start=(t == 0), stop=(t == nrows - 1))
                else:
                    sum_acc = stat.tile([128, free], mybir.dt.float32, tag="sum")
                    ssq_acc = stat.tile([128, free], mybir.dt.float32, tag="ssq")
                    nc.vector.tensor_copy(sum_acc, ld[:, 0, :])
                    for t in range(1, nrows):
                        nc.vector.tensor_tensor(out=sum_acc, in0=sum_acc,
                                                in1=ld[:, t, :], op=mybir.AluOpType.add)
                    nc.vector.tensor_copy(ssq_acc, sq[:, 0, :])
                    for t in range(1, nrows):
                        nc.vector.tensor_tensor(out=ssq_acc, in0=ssq_acc,
                                                in1=sq[:, t, :], op=mybir.AluOpType.add)
                t1 = stat.tile([128, free], mybir.dt.float32, tag="t1")
                # sum^2 via ACT Square: single PSUM read, exact x*x
                nc.scalar.activation(out=t1, in_=sum_acc,
                                     func=mybir.ActivationFunctionType.Square)
                nc.vector.scalar_tensor_tensor(
                    out=t1, in0=t1, scalar=-0.1, in1=ssq_acc,
                    op0=mybir.AluOpType.mult, op1=mybir.AluOpType.add,
                )
                nc.sync.dma_start(out=qvar[:, it * free:(it + 1) * free], in_=t1)
    if split_waits:
        _split_sync_waits(nc)
    return nc


def build_l2(slice_free=SLICE_FREE, levels=LEVELS_Q, rank=RANK,
             split_waits=True):
    """L2: median of one q channel per core."""
    import concourse.bass as bass
    import concourse.tile as tile
    from concourse import mybir

    _apply_tile_patch()
    nc = bass.Bass("TRN2", target_bir_lowering=False, debug=False, num_devices=1)
    qslice = nc.dram_tensor("qslice", [1, 128 * slice_free], mybir.dt.float32,
                            kind="ExternalInput").ap()
    qrange = nc.dram_tensor("qrange", [1, 2], mybir.dt.float32,
                            kind="ExternalInput").ap()
    qmed = nc.dram_tensor("qmed", [1, 1], mybir.dt.float32,
                          kind="ExternalOutput").ap()

    with tile.TileContext(nc) as tc:
        with tc.tile_pool(name="sl", bufs=1) as slpool, \
             tc.tile_pool(name="small", bufs=1) as small, \
             tc.tile_pool(name="ps", bufs=2, space="PSUM") as psum:
            ones = small.tile([128, 128], mybir.dt.float32)
            nc.vector.memset(ones, 1.0)
            junk = small.tile([128, slice_free], mybir.dt.bfloat16)
            d = slpool.tile([128, slice_free], mybir.dt.float32)
            nc.sync.dma_start(out=d, in_=qslice[0].rearrange("(p f) -> p f", p=128))
            st = small.tile([128, 8], mybir.dt.float32)
            nc.vector.memset(st, 0.0)
            nc.sync.dma_start(
                out=st[:, 0:2],
                in_=bass.AP(tensor=qrange.tensor, offset=0, ap=[[0, 128], [1, 2]]),
            )
            _bisect_median(nc, small, psum, ones, [d], [st], junk, levels, rank,
                           n_padded=128 * slice_free)
            medt = small.tile([1, 1], mybir.dt.float32)
            nc.vector.tensor_tensor(out=medt, in0=st[0:1, 0:1], in1=st[0:1, 1:2],
                                    op=mybir.AluOpType.add)
            nc.sync.dma_start(out=qmed, in_=medt)
    if split_waits:
        _split_sync_waits(nc)
    return nc


def build_l3(padw_pp=PADW_PP, nrows=5, need=3.0, split_waits=True):
    """L3: mode shard = (#rows with x >= th[t,ch]) >= need."""
    import concourse.bass as bass
    import concourse.tile as tile
    from concourse import mybir

    _apply_tile_patch()
    chunk_pp = padw_pp // 8
    qw = padw_pp * 4
    nc = bass.Bass("TRN2", target_bir_lowering=False, debug=False, num_devices=1)
    shard = nc.dram_tensor("shardpad", [10, 128 * padw_pp, 4], mybir.dt.float32,
                           kind="ExternalInput").ap()
    th = nc.dram_tensor("th", [nrows, 4], mybir.dt.float32,
                        kind="ExternalInput").ap()
    modeo = nc.dram_tensor("mode", [128, qw], mybir.dt.float32,
                           kind="ExternalOutput").ap()

    with tile.TileContext(nc) as tc:
        with tc.tile_pool(name="stream", bufs=3) as stream, \
             tc.tile_pool(name="acc", bufs=2) as accpool, \
             tc.tile_pool(name="small", bufs=1) as small:
            thb = small.tile([128, nrows * 4], mybir.dt.float32)
            nc.sync.dma_start(
                out=thb,
                in_=bass.AP(tensor=th.tensor, offset=0, ap=[[0, 128], [1, nrows * 4]]),
            )
            free = chunk_pp * 4
            for it in range(8):
                ld = stream.tile([128, nrows, free], mybir.dt.float32, tag="ld")
                src = bass.AP(
                    tensor=shard.tensor,
                    offset=it * chunk_pp * 4,
                    ap=[[padw_pp * 4, 128], [128 * padw_pp * 4, nrows],
                        [4, chunk_pp], [1, 4]],
                )
                nc.sync.dma_start(out=ld.rearrange("p t (c k) -> p t c k", k=4), in_=src)
                acc = accpool.tile([128, free], mybir.dt.float32, tag="acc")
                cmp = accpool.tile([128, free], mybir.dt.float32, tag="cmp")
                for t in range(nrows):
                    thv = bass.AP(tensor=thb.tensor, offset=thb.offset + t * 4,
                                  ap=[thb.ap[0], [0, chunk_pp], [1, 4]])
                    dst = acc if t == 0 else cmp
                    nc.vector.scalar_tensor_tensor(
                        out=dst.rearrange("p (c k) -> p c k", k=4),
                        in0=thv, scalar=0.0,
                        in1=ld[:, t, :].rearrange("p (c k) -> p c k", k=4),
                        op0=mybir.AluOpType.add, op1=mybir.AluOpType.is_le,
                    )
                    if t > 0:
                        nc.vector.tensor_tensor(out=acc, in0=acc, in1=cmp,
                                                op=mybir.AluOpType.add)
                mch = accpool.tile([128, free], mybir.dt.float32, tag="mch")
                nc.vector.tensor_scalar(out=mch, in0=acc, scalar1=need, scalar2=None,
                                        op0=mybir.AluOpType.is_ge)
                nc.sync.dma_start(out=modeo[:, it * free:(it + 1) * free], in_=mch)
    if split_waits:
        _split_sync_waits(nc)
    return nc


def _pad_shard(logits_shard, padw_pp=PADW_PP):
    """(10, SHARD, 4) -> (10, 128*padw_pp, 4) zero-padded."""
    nrows, w, chn = logits_shard.shape
    out = np.zeros((nrows, 128 * padw_pp, chn), dtype=np.float32)
    out[:, :w, :] = logits_shard
    return out


def _pad_slice(v, slice_free=SLICE_FREE):
    out = np.full(128 * slice_free, PAD_BIG, dtype=np.float32)
    out[: v.shape[0]] = v
    return out


def _trim(arr128, width, padw_pp=PADW_PP):
    """[128, padw_pp*4] core output -> (width, 4)."""
    return arr128.reshape(128 * padw_pp, 4)[:width]


def _logsumexp_f32(v):
    m = np.max(v)
    return np.float32(np.log(np.sum(np.exp(v - m, dtype=np.float32), dtype=np.float32)) + m)


def _numpy_fallback(logits, x, delta):
    logits = np.asarray(logits, dtype=np.float32)
    x = np.asarray(x, dtype=np.float32)
    delta = np.float32(delta)
    n = logits.shape[1]
    med = np.sort(logits, axis=1)[:, (n - 1) // 2, :]
    std = np.asarray(logits, dtype=np.float32).std(axis=0, ddof=1).astype(np.float32)
    std_med = np.sort(std, axis=0)[(n - 1) // 2, :]
    thresh = med[:, None, :]
    above = (logits >= thresh + FACTOR * std_med) & (logits >= thresh + delta / 2)
    cls = above.astype(np.int32)
    s = cls[:5].sum(axis=0)
    mode = (s >= 3).astype(np.float32)
    c = np.broadcast_to(mode[None], logits.shape).astype(np.float32)
    xs = np.concatenate([np.zeros((x.shape[0], 1), x.dtype), x], axis=1)
    dx = delta * c + xs[:, None, :]
    outs = []
    for i in range(4):
        oth = [j for j in range(4) if j != i]
        m = dx[..., oth].max(axis=-1)
        lse = np.log(np.sum(np.exp(dx[..., oth] - m[..., None]), axis=-1)) + m
        outs.append(dx[..., i] - lse)
    return np.stack(outs, axis=-1).astype(np.float32), c


def kernel(logits, x, delta):
    logits = np.ascontiguousarray(np.asarray(logits, dtype=np.float32))
    x = np.asarray(x, dtype=np.float32)
    dval = float(np.asarray(delta))
    if dval != 0.0 or logits.shape != (10, N, 4):
        return _numpy_fallback(logits, x, delta)

    from concourse.bass_utils import run_bass_kernel_spmd

    def _run(nc, in_maps, cores):
        # a wedged accelerator session recovers on a fresh NRT attempt
        import time as _t
        try:
            return run_bass_kernel_spmd(nc, in_maps, core_ids=cores)
        except Exception:
            _t.sleep(5)
            return run_bass_kernel_spmd(nc, in_maps, core_ids=cores)

    cores = list(range(NCORES))

    # ---------- launch 1: stats + logits medians ----------
    slice_assign = [(t, ch) for t in range(5) for ch in range(4)]
    slice_assign += [(0, 0)] * (3 * NCORES - len(slice_assign))  # dummy slots
    shard_pads = []
    in1 = []
    for c in cores:
        sh = _pad_shard(logits[:, c * SHARD:(c + 1) * SHARD, :])
        shard_pads.append(sh)
        sl = np.stack([
            _pad_slice(logits[t, :, ch]) for (t, ch) in slice_assign[3 * c:3 * c + 3]
        ])
        rg = np.array([[MED_RANGE[0], (MED_RANGE[1] - MED_RANGE[0]) / 2]] * 3,
                      dtype=np.float32)
        in1.append({"shardpad": sh, "slices": sl, "ranges": rg,
                    "ident": np.eye(128, dtype=np.float32)})
    import time as _time
    nc1 = build_l1()
    _t = _time.time()
    r1 = _run(nc1, in1, cores)
    LAST_RUN_TIMES.append(_time.time() - _t)

    qvar = np.concatenate(
        [_trim(r1.results[c]["qvar"], SHARD) for c in cores], axis=0
    )  # (N, 4)
    med = np.zeros((5, 4), dtype=np.float32)
    med_margin = 4 * (MED_RANGE[1] - MED_RANGE[0]) / 2 ** LEVELS_MED
    for idx, (t, ch) in enumerate(slice_assign[:20]):
        m = r1.results[idx // 3]["med"][0, idx % 3]
        if not (MED_RANGE[0] + med_margin < m < MED_RANGE[1] - med_margin):
            # bracket miss (never for N(0,1) inputs): exact host re-derivation
            m = np.partition(logits[t, :, ch], (N - 1) // 2)[(N - 1) // 2]
        med[t, ch] = m

    # ---------- launch 2: q medians per channel ----------
    in2 = []
    for c in cores:
        ch = c % 4
        in2.append({
            "qslice": _pad_slice(qvar[:, ch])[None, :],
            "qrange": np.array([[Q_RANGE[0], (Q_RANGE[1] - Q_RANGE[0]) / 2]],
                               dtype=np.float32),
        })
    nc2 = build_l2()
    _t = _time.time()
    r2 = _run(nc2, in2, cores)
    LAST_RUN_TIMES.append(_time.time() - _t)
    q_margin = 4 * (Q_RANGE[1] - Q_RANGE[0]) / 2 ** LEVELS_Q
    qmed = np.zeros(4, dtype=np.float32)
    for ch in range(4):
        qm = r2.results[ch]["qmed"][0, 0]
        if not (Q_RANGE[0] + q_margin < qm < Q_RANGE[1] - q_margin):
            qm = np.partition(qvar[:, ch], (N - 1) // 2)[(N - 1) // 2]
        qmed[ch] = qm
    std_med = np.sqrt(qmed / np.float32(9)).astype(np.float32)

    # ---------- launch 3: mode ----------
    th = (med + FACTOR * std_med[None, :]).astype(np.float32)
    in3 = [{"shardpad": shard_pads[c], "th": th} for c in cores]
    nc3 = build_l3()
    _t = _time.time()
    r3 = _run(nc3, in3, cores)
    LAST_RUN_TIMES.append(_time.time() - _t)
    mode = np.concatenate(
        [_trim(r3.results[c]["mode"], SHARD) for c in cores], axis=0
    )  # (N, 4) of 0.0/1.0

    # ---------- host assembly ----------
    xs = np.concatenate([np.zeros((x.shape[0], 1), np.float32), x], axis=1)
    table = np.zeros((10, 4), dtype=np.float32)
    for t in range(10):
        for i in range(4):
            oth = [j for j in range(4) if j != i]
            table[t, i] = xs[t, i] - _logsumexp_f32(xs[t, oth])
    out_full = np.broadcast_to(table[:, None, :], (10, N, 4))
    c_full = np.broadcast_to(mode[None], (10, N, 4))
    return out_full, c_full



# revision 2
# speedup vs baseline: 2.8607x; 2.8607x over previous
"""Trainium2 Bass kernel for nn_DeltaModel (histogram_binning).

Reference semantics (delta == 0, the shipped configuration):
  med[t,ch]   = lower median over N of logits[t,:,ch]   (only rows 0-4 used)
  q[n,ch]     = sumsq - 0.1*sum^2 over the 10 rows      (q/9 = unbiased var)
  std_med[ch] = sqrt(median_N(q[:,ch]) / 9)
  mode[n,ch]  = (#{t<5: logits[t,n,ch] >= med[t,ch] + 1.96*std_med[ch]} >= 3)
              = (median5_t(logits[t,n,ch] - med[t,ch]) >= 1.96*std_med[ch])
  c           = broadcast(mode) over dim 0
  out[t,:,ch] = xs[t,ch] - logsumexp(xs[t,others(ch)])  (constant over N)

Single SPMD launch over 8 NeuronCores (one compile, one 160MB transfer):
each core streams its column shard once, producing the per-column q and
the per-column median-of-5 of the med-shifted rows 0-4 (m3).  The host
supplies exact med[t,ch] (np.partition), then finishes with the q median
(np.partition), the m3 >= 1.96*std_med threshold, and broadcast assembly.
"""

import numpy as np

LAST_RUN_TIMES = []  # wall seconds of each device launch (incl. first-call compile)

N = 1_000_000
NCORES = 8
SHARD = N // NCORES            # 125000
PADW_PP = 992                  # per-partition padded columns (125000/128 -> 976.6)
SHARD_PAD = 128 * PADW_PP      # 126976
NCHUNK = 4
FACTOR = np.float32(1.96)


def _apply_tile_patch():
    """This walrus build rejects >2 sync waits on the SP Drain emitted at
    TileContext exit ("Too many sync wait commands"); keep one wait on the
    drain and move the rest onto dedicated SP nops before the barrier."""
    import concourse.tile as tile_mod
    from concourse import mybir
    from concourse.vector_clock import ScopedClock

    if getattr(tile_mod.TileContext, "_ant_drain_patched", False):
        return

    def _patched(self, tick_clock, wait_clock):
        nc = self.nc
        drain_inst = nc.sync.drain()
        wait_clock.add_sem_waits(
            drain_inst.ins, ScopedClock({None: tick_clock.global_clock})
        )
        si = drain_inst.ins.sync_info
        if si is not None and si.on_wait is not None and len(si.on_wait) > 1:
            waits = list(si.on_wait)
            drain_inst.ins.sync_info = mybir.SyncInfo(
                on_wait=waits[:1], on_update=list(si.on_update or [])
            )
            for w in waits[1:]:
                nop = nc.sync.nop()
                nop.ins.sync_info = mybir.SyncInfo(on_wait=[w], on_update=[])
        nc.all_engine_barrier()
        assert self.sems is not None
        popped = nc._tile_sem_poison_stack.pop()
        assert popped is self._sem_poison
        nc.clear_and_free_semaphores(list(self.sems.allocated().values()))
        nc.all_engine_barrier()

    tile_mod.TileContext._drain_and_barrier = _patched
    tile_mod.TileContext._ant_drain_patched = True


def _split_sync_waits(nc, maxw=1):
    """This walrus build caps per-instruction sync waits; move excess waits
    onto same-engine NoOps inserted right before the offending instruction."""
    from concourse import mybir

    for f in nc.m.functions:
        for b in f.blocks:
            new_list = []
            changed = False
            for ins in b.instructions:
                si = getattr(ins, "sync_info", None)
                if si is not None and si.on_wait and len(si.on_wait) > maxw:
                    waits = list(si.on_wait)
                    extra, keep = waits[:-maxw], waits[-maxw:]
                    for i in range(0, len(extra), maxw):
                        nop = mybir.InstNoOp(
                            name=f"{ins.name}-wsplit{i}", ins=[], outs=[]
                        )
                        nop.engine = ins.engine
                        nop.sync_info = mybir.SyncInfo(
                            on_wait=extra[i:i + maxw], on_update=[]
                        )
                        new_list.append(nop)
                        changed = True
                    ins.sync_info = mybir.SyncInfo(
                        on_wait=keep, on_update=list(si.on_update or [])
                    )
                new_list.append(ins)
            if changed:
                b.instructions = new_list


def build_fused(padw_pp=PADW_PP, nchunk=NCHUNK, nrows=10, nmed=5,
                split_waits=True):
    """One pass over the shard: q = sumsq - 0.1*sum^2 over the 10 rows and
    m3 = median-of-5 of (row_t - med[t]) for rows 0-4, both per column."""
    import concourse.bass as bass
    import concourse.tile as tile
    from concourse import mybir

    _apply_tile_patch()
    chunk_pp = padw_pp // nchunk
    qw = padw_pp * 4
    F = chunk_pp * 4                   # free elements per chunk (ch-interleaved)
    nc = bass.Bass("TRN2", target_bir_lowering=False, debug=False, num_devices=1)
    shard = nc.dram_tensor("shardpad", [nrows, 128 * padw_pp, 4], mybir.dt.float32,
                           kind="ExternalInput").ap()
    medin = nc.dram_tensor("medin", [nmed, 4], mybir.dt.float32,
                           kind="ExternalInput").ap()
    qvar = nc.dram_tensor("qvar", [128, qw], mybir.dt.float32,
                          kind="ExternalOutput").ap()
    m3o = nc.dram_tensor("m3", [128, qw], mybir.dt.float32,
                         kind="ExternalOutput").ap()

    with tile.TileContext(nc) as tc:
        with tc.tile_pool(name="stream", bufs=2) as stream, \
             tc.tile_pool(name="scr", bufs=2) as scr, \
             tc.tile_pool(name="small", bufs=1) as small:
            medb = small.tile([128, nmed * 4], mybir.dt.float32)
            nc.sync.dma_start(
                out=medb,
                in_=bass.AP(tensor=medin.tensor, offset=0,
                            ap=[[0, 128], [1, nmed * 4]]),
            )
            for it in range(nchunk):
                ld = stream.tile([128, nrows, F], mybir.dt.float32, tag="ld")
                src = bass.AP(
                    tensor=shard.tensor,
                    offset=it * chunk_pp * 4,
                    ap=[[padw_pp * 4, 128], [128 * padw_pp * 4, nrows],
                        [4, chunk_pp], [1, 4]],
                )
                nc.sync.dma_start(out=ld.rearrange("p t (c k) -> p t c k", k=4),
                                  in_=src)
                # ---- q over all 10 rows ----
                sumt = scr.tile([128, F], mybir.dt.float32, tag="sum", name="sum")
                ssq = scr.tile([128, F], mybir.dt.float32, tag="ssq", name="ssq")
                sq = scr.tile([128, F], mybir.dt.float32, tag="sq", name="sq")
                nc.vector.tensor_copy(sumt, ld[:, 0, :])
                nc.scalar.activation(out=ssq, in_=ld[:, 0, :],
                                     func=mybir.ActivationFunctionType.Square)
                for t in range(1, nrows):
                    nc.scalar.activation(out=sq, in_=ld[:, t, :],
                                         func=mybir.ActivationFunctionType.Square)
                    nc.vector.tensor_tensor(out=sumt, in0=sumt, in1=ld[:, t, :],
                                            op=mybir.AluOpType.add)
                    nc.vector.tensor_tensor(out=ssq, in0=ssq, in1=sq,
                                            op=mybir.AluOpType.add)
                nc.scalar.activation(out=sq, in_=sumt,
                                     func=mybir.ActivationFunctionType.Square)
                nc.vector.scalar_tensor_tensor(
                    out=ssq, in0=sq, scalar=-0.1, in1=ssq,
                    op0=mybir.AluOpType.mult, op1=mybir.AluOpType.add,
                )
                nc.sync.dma_start(out=qvar[:, it * F:(it + 1) * F], in_=ssq)

                # ---- m3 over med-shifted rows 0-4 (in-place on ld) ----
                for t in range(nmed):
                    medv = bass.AP(tensor=medb.tensor, offset=medb.offset + t * 4,
                                   ap=[medb.ap[0], [0, chunk_pp], [1, 4]])
                    nc.vector.scalar_tensor_tensor(
                        out=ld[:, t, :].rearrange("p (c k) -> p c k", k=4),
                        in0=medv, scalar=-1.0,
                        in1=ld[:, t, :].rearrange("p (c k) -> p c k", k=4),
                        op0=mybir.AluOpType.mult, op1=mybir.AluOpType.add,
                    )
                y = [ld[:, t, :] for t in range(nmed)]
                s1 = scr.tile([128, F], mybir.dt.float32, tag="s1", name="s1")
                s2 = scr.tile([128, F], mybir.dt.float32, tag="s2", name="s2")
                mx = mybir.AluOpType.max
                mn = mybir.AluOpType.min
                tt = nc.vector.tensor_tensor
                tt(out=s1, in0=y[0], in1=y[1], op=mx)    # s1 = max01
                tt(out=y[0], in0=y[0], in1=y[1], op=mn)  # y0 = min01
                tt(out=s2, in0=y[2], in1=y[3], op=mx)    # s2 = max23
                tt(out=y[2], in0=y[2], in1=y[3], op=mn)  # y2 = min23
                tt(out=y[0], in0=y[0], in1=y[2], op=mx)  # f = max(min01, min23)
                tt(out=s1, in0=s1, in1=s2, op=mn)        # g = min(max01, max23)
                tt(out=s2, in0=y[4], in1=y[0], op=mx)    # v = max(e, f)
                tt(out=y[4], in0=y[4], in1=y[0], op=mn)  # u = min(e, f)
                tt(out=s2, in0=s2, in1=s1, op=mn)        # w = min(v, g)
                tt(out=s2, in0=y[4], in1=s2, op=mx)      # m3 = max(u, w)
                nc.sync.dma_start(out=m3o[:, it * F:(it + 1) * F], in_=s2)
    if split_waits:
        _split_sync_waits(nc)
    return nc


def _pad_shard(logits_shard, padw_pp=PADW_PP):
    """(10, SHARD, 4) -> (10, 128*padw_pp, 4) zero-padded."""
    nrows, w, chn = logits_shard.shape
    out = np.zeros((nrows, 128 * padw_pp, chn), dtype=np.float32)
    out[:, :w, :] = logits_shard
    return out


def _trim(arr128, width, padw_pp=PADW_PP):
    """[128, padw_pp*4] core output -> (width, 4)."""
    return arr128.reshape(128 * padw_pp, 4)[:width]


def _logsumexp_f32(v):
    m = np.max(v)
    return np.float32(np.log(np.sum(np.exp(v - m, dtype=np.float32), dtype=np.float32)) + m)


def _numpy_fallback(logits, x, delta):
    logits = np.asarray(logits, dtype=np.float32)
    x = np.asarray(x, dtype=np.float32)
    delta = np.float32(delta)
    n = logits.shape[1]
    med = np.sort(logits, axis=1)[:, (n - 1) // 2, :]
    std = np.asarray(logits, dtype=np.float32).std(axis=0, ddof=1).astype(np.float32)
    std_med = np.sort(std, axis=0)[(n - 1) // 2, :]
    thresh = med[:, None, :]
    above = (logits >= thresh + FACTOR * std_med) & (logits >= thresh + delta / 2)
    cls = above.astype(np.int32)
    s = cls[:5].sum(axis=0)
    mode = (s >= 3).astype(np.float32)
    c = np.broadcast_to(mode[None], logits.shape).astype(np.float32)
    xs = np.concatenate([np.zeros((x.shape[0], 1), x.dtype), x], axis=1)
    dx = delta * c + xs[:, None, :]
    outs = []
    for i in range(4):
        oth = [j for j in range(4) if j != i]
        m = dx[..., oth].max(axis=-1)
        lse = np.log(np.sum(np.exp(dx[..., oth] - m[..., None]), axis=-1)) + m
        outs.append(dx[..., i] - lse)
    return np.stack(outs, axis=-1).astype(np.float32), c


def _median_lower(v):
    """Exact torch-style lower median of a 1D f32 array."""
    k = (v.shape[0] - 1) // 2
    return np.partition(v, k)[k]


def kernel(logits, x, delta):
    logits = np.ascontiguousarray(np.asarray(logits, dtype=np.float32))
    x = np.asarray(x, dtype=np.float32)
    dval = float(np.asarray(delta))
    if dval != 0.0 or logits.shape != (10, N, 4):
        return _numpy_fallback(logits, x, delta)

    from concourse.bass_utils import run_bass_kernel_spmd

    def _run(nc, in_maps, cores):
        # a wedged accelerator session recovers on a fresh NRT attempt
        import time as _t
        try:
            return run_bass_kernel_spmd(nc, in_maps, core_ids=cores)
        except Exception:
            _t.sleep(5)
            return run_bass_kernel_spmd(nc, in_maps, core_ids=cores)

    cores = list(range(NCORES))

    # exact lower medians of rows 0-4 on host (the only cross-shard quantile
    # besides qmed below); ~150ms of np.partition over 80MB
    med = np.empty((5, 4), dtype=np.float32)
    for t in range(5):
        for ch in range(4):
            med[t, ch] = _median_lower(np.ascontiguousarray(logits[t, :, ch]))

    in1 = [{"shardpad": _pad_shard(logits[:, c * SHARD:(c + 1) * SHARD, :]),
            "medin": med} for c in cores]
    import time as _time
    nc1 = build_fused()
    _t = _time.time()
    r1 = _run(nc1, in1, cores)
    LAST_RUN_TIMES.append(_time.time() - _t)

    qvar = np.concatenate(
        [_trim(r1.results[c]["qvar"], SHARD) for c in cores], axis=0
    )  # (N, 4)
    m3 = np.concatenate(
        [_trim(r1.results[c]["m3"], SHARD) for c in cores], axis=0
    )  # (N, 4)

    qmed = np.empty(4, dtype=np.float32)
    for ch in range(4):
        qmed[ch] = _median_lower(np.ascontiguousarray(qvar[:, ch]))
    std_med = np.sqrt(qmed / np.float32(9)).astype(np.float32)

    mode = (m3 >= FACTOR * std_med[None, :]).astype(np.float32)  # (N, 4)

    # ---- host assembly ----
    xs = np.concatenate([np.zeros((x.shape[0], 1), np.float32), x], axis=1)
    table = np.zeros((10, 4), dtype=np.float32)
    for t in range(10):
        for i in range(4):
            oth = [j for j in range(4) if j != i]
            table[t, i] = xs[t, i] - _logsumexp_f32(xs[t, oth])
    out_full = np.broadcast_to(table[:, None, :], (10, N, 4))
    c_full = np.broadcast_to(mode[None], (10, N, 4))
    return out_full, c_full


# revision 3
# speedup vs baseline: 3.6335x; 1.2701x over previous
"""Trainium2 Bass kernel for nn_DeltaModel (histogram_binning).

Reference semantics (delta == 0, the shipped configuration):
  med[t,ch]   = lower median over N of logits[t,:,ch]   (only rows 0-4 used)
  q[n,ch]     = sumsq - 0.1*sum^2 over the 10 rows      (q/9 = unbiased var)
  std_med[ch] = sqrt(median_N(q[:,ch]) / 9)
  mode[n,ch]  = (#{t<5: logits[t,n,ch] >= med[t,ch] + 1.96*std_med[ch]} >= 3)
              = (median5_t(logits[t,n,ch] - med[t,ch]) >= 1.96*std_med[ch])
  c           = broadcast(mode) over dim 0
  out[t,:,ch] = xs[t,ch] - logsumexp(xs[t,others(ch)])  (constant over N)

Single SPMD launch over 8 NeuronCores (one compile, one transfer): each
core streams its column shard once, producing per-column q (as q-8.35 in
f16) and the median-of-5 of the med-shifted rows 0-4 (m3, f16).  The
transfer link (~60MB/s) dominates, so the shard ships as float16: the
q median only moves ~1e-5 (well under the ~1e-5 std_med budget), and m3's
<=3.4e-3 quantization error is absorbed by an exact host re-check of the
~50 columns that land within `margin` of the threshold.  The host
supplies exact med[t,ch] (np.partition), finishes qmed by partition,
thresholds m3, re-checks the ambiguous columns against the full-precision
logits it already holds, and assembles the broadcast outputs.
"""

import numpy as np

LAST_RUN_TIMES = []  # wall seconds of each device launch (incl. first-call compile)

N = 1_000_000
NCORES = 8
SHARD = N // NCORES            # 125000
PADW_PP = 992                  # per-partition padded columns
SHARD_PAD = 128 * PADW_PP      # 126976
NCHUNK = 2
FACTOR = np.float32(1.96)
Q_OFF = np.float32(8.35)       # chi^2_9 median ~8.34; q output is q - Q_OFF
Q_BRACKET = 0.45               # qmed must land in (Q_OFF-0.45, Q_OFF+0.45)


def _apply_tile_patch():
    """This walrus build rejects >2 sync waits on the SP Drain emitted at
    TileContext exit ("Too many sync wait commands"); keep one wait on the
    drain and move the rest onto dedicated SP nops before the barrier."""
    import concourse.tile as tile_mod
    from concourse import mybir
    from concourse.vector_clock import ScopedClock

    if getattr(tile_mod.TileContext, "_ant_drain_patched", False):
        return

    def _patched(self, tick_clock, wait_clock):
        nc = self.nc
        drain_inst = nc.sync.drain()
        wait_clock.add_sem_waits(
            drain_inst.ins, ScopedClock({None: tick_clock.global_clock})
        )
        si = drain_inst.ins.sync_info
        if si is not None and si.on_wait is not None and len(si.on_wait) > 1:
            waits = list(si.on_wait)
            drain_inst.ins.sync_info = mybir.SyncInfo(
                on_wait=waits[:1], on_update=list(si.on_update or [])
            )
            for w in waits[1:]:
                nop = nc.sync.nop()
                nop.ins.sync_info = mybir.SyncInfo(on_wait=[w], on_update=[])
        nc.all_engine_barrier()
        assert self.sems is not None
        popped = nc._tile_sem_poison_stack.pop()
        assert popped is self._sem_poison
        nc.clear_and_free_semaphores(list(self.sems.allocated().values()))
        nc.all_engine_barrier()

    tile_mod.TileContext._drain_and_barrier = _patched
    tile_mod.TileContext._ant_drain_patched = True


def _split_sync_waits(nc, maxw=1):
    """This walrus build caps per-instruction sync waits; move excess waits
    onto same-engine NoOps inserted right before the offending instruction."""
    from concourse import mybir

    for f in nc.m.functions:
        for b in f.blocks:
            new_list = []
            changed = False
            for ins in b.instructions:
                si = getattr(ins, "sync_info", None)
                if si is not None and si.on_wait and len(si.on_wait) > maxw:
                    waits = list(si.on_wait)
                    extra, keep = waits[:-maxw], waits[-maxw:]
                    for i in range(0, len(extra), maxw):
                        nop = mybir.InstNoOp(
                            name=f"{ins.name}-wsplit{i}", ins=[], outs=[]
                        )
                        nop.engine = ins.engine
                        nop.sync_info = mybir.SyncInfo(
                            on_wait=extra[i:i + maxw], on_update=[]
                        )
                        new_list.append(nop)
                        changed = True
                    ins.sync_info = mybir.SyncInfo(
                        on_wait=keep, on_update=list(si.on_update or [])
                    )
                new_list.append(ins)
            if changed:
                b.instructions = new_list


def build_fused(padw_pp=PADW_PP, nchunk=NCHUNK, nrows=10, nmed=5,
                split_waits=True):
    """One pass over the f16 shard: q-8.35 (f16) over the 10 rows and m3 =
    median-of-5 of (row_t - med[t]) for rows 0-4 (f16), both per column."""
    import concourse.bass as bass
    import concourse.tile as tile
    from concourse import mybir

    _apply_tile_patch()
    chunk_pp = padw_pp // nchunk
    qw = padw_pp * 4
    F = chunk_pp * 4                   # free elements per chunk (ch-interleaved)
    nc = bass.Bass("TRN2", target_bir_lowering=False, debug=False, num_devices=1)
    shard = nc.dram_tensor("shardf16", [nrows, 128 * padw_pp, 4], mybir.dt.float16,
                           kind="ExternalInput").ap()
    medin = nc.dram_tensor("medin", [nmed, 4], mybir.dt.float32,
                           kind="ExternalInput").ap()
    qo = nc.dram_tensor("qo", [128, qw], mybir.dt.float16,
                        kind="ExternalOutput").ap()
    m3o = nc.dram_tensor("m3", [128, qw], mybir.dt.float16,
                         kind="ExternalOutput").ap()

    with tile.TileContext(nc) as tc:
        with tc.tile_pool(name="stream", bufs=2) as stream, \
             tc.tile_pool(name="scr", bufs=1) as scr, \
             tc.tile_pool(name="small", bufs=1) as small:
            medb = small.tile([128, nmed * 4], mybir.dt.float32)
            nc.sync.dma_start(
                out=medb,
                in_=bass.AP(tensor=medin.tensor, offset=0,
                            ap=[[0, 128], [1, nmed * 4]]),
            )
            mx = mybir.AluOpType.max
            mn = mybir.AluOpType.min
            add = mybir.AluOpType.add
            for it in range(nchunk):
                ld = stream.tile([128, nrows, F], mybir.dt.float16, tag="ld")
                src = bass.AP(
                    tensor=shard.tensor,
                    offset=it * chunk_pp * 4,
                    ap=[[padw_pp * 4, 128], [128 * padw_pp * 4, nrows],
                        [4, chunk_pp], [1, 4]],
                )
                nc.sync.dma_start(out=ld.rearrange("p t (c k) -> p t c k", k=4),
                                  in_=src)
                # rows 0-4 upcast once; reused by both q and m3 phases
                y = [scr.tile([128, F], mybir.dt.float32, tag=f"y{t}", name=f"y{t}")
                     for t in range(nmed)]
                for t in range(nmed):
                    nc.vector.tensor_copy(y[t], ld[:, t, :])
                # ---- q over all 10 rows ----
                sumt = scr.tile([128, F], mybir.dt.float32, tag="sum", name="sum")
                ssq = scr.tile([128, F], mybir.dt.float32, tag="ssq", name="ssq")
                sq = scr.tile([128, F], mybir.dt.float32, tag="sq", name="sq")
                xf = scr.tile([128, F], mybir.dt.float32, tag="xf", name="xf")
                nc.vector.tensor_copy(sumt, y[0])
                nc.scalar.activation(out=ssq, in_=y[0],
                                     func=mybir.ActivationFunctionType.Square)
                for t in range(1, nrows):
                    if t < nmed:
                        xt = y[t]
                    else:
                        nc.vector.tensor_copy(xf, ld[:, t, :])
                        xt = xf
                    nc.scalar.activation(out=sq, in_=xt,
                                         func=mybir.ActivationFunctionType.Square)
                    nc.vector.tensor_tensor(out=sumt, in0=sumt, in1=xt, op=add)
                    nc.vector.tensor_tensor(out=ssq, in0=ssq, in1=sq, op=add)
                nc.scalar.activation(out=sq, in_=sumt,
                                     func=mybir.ActivationFunctionType.Square)
                nc.vector.scalar_tensor_tensor(
                    out=ssq, in0=sq, scalar=-0.1, in1=ssq,
                    op0=mybir.AluOpType.mult, op1=add,
                )
                q16 = scr.tile([128, F], mybir.dt.float16, tag="q16", name="q16")
                nc.vector.tensor_scalar(out=q16, in0=ssq, scalar1=-float(Q_OFF),
                                        scalar2=None, op0=add)
                nc.sync.dma_start(out=qo[:, it * F:(it + 1) * F], in_=q16)

                # ---- m3 over med-shifted rows 0-4 (in-place on y) ----
                for t in range(nmed):
                    medv = bass.AP(tensor=medb.tensor, offset=medb.offset + t * 4,
                                   ap=[medb.ap[0], [0, chunk_pp], [1, 4]])
                    nc.vector.scalar_tensor_tensor(
                        out=y[t].rearrange("p (c k) -> p c k", k=4),
                        in0=medv, scalar=-1.0,
                        in1=y[t].rearrange("p (c k) -> p c k", k=4),
                        op0=mybir.AluOpType.mult, op1=add,
                    )
                s1 = scr.tile([128, F], mybir.dt.float32, tag="s1", name="s1")
                s2 = scr.tile([128, F], mybir.dt.float32, tag="s2", name="s2")
                m16 = scr.tile([128, F], mybir.dt.float16, tag="m16", name="m16")
                tt = nc.vector.tensor_tensor
                tt(out=s1, in0=y[0], in1=y[1], op=mx)    # s1 = max01
                tt(out=y[0], in0=y[0], in1=y[1], op=mn)  # y0 = min01
                tt(out=s2, in0=y[2], in1=y[3], op=mx)    # s2 = max23
                tt(out=y[2], in0=y[2], in1=y[3], op=mn)  # y2 = min23
                tt(out=y[0], in0=y[0], in1=y[2], op=mx)  # f = max(min01, min23)
                tt(out=s1, in0=s1, in1=s2, op=mn)        # g = min(max01, max23)
                tt(out=s2, in0=y[4], in1=y[0], op=mx)    # v = max(e, f)
                tt(out=y[4], in0=y[4], in1=y[0], op=mn)  # u = min(e, f)
                tt(out=s2, in0=s2, in1=s1, op=mn)        # w = min(v, g)
                tt(out=m16, in0=y[4], in1=s2, op=mx)     # m3 = max(u, w)
                nc.sync.dma_start(out=m3o[:, it * F:(it + 1) * F], in_=m16)
    if split_waits:
        _split_sync_waits(nc)
    return nc


def _pad_shard16(lf16_shard, padw_pp=PADW_PP):
    """(10, SHARD, 4) f16 -> (10, 128*padw_pp, 4) zero-padded."""
    nrows, w, chn = lf16_shard.shape
    out = np.zeros((nrows, 128 * padw_pp, chn), dtype=np.float16)
    out[:, :w, :] = lf16_shard
    return out


def _trim(arr128, width, padw_pp=PADW_PP):
    """[128, padw_pp*4] core output -> (width, 4)."""
    return arr128.reshape(128 * padw_pp, 4)[:width]


def _logsumexp_f32(v):
    m = np.max(v)
    return np.float32(np.log(np.sum(np.exp(v - m, dtype=np.float32), dtype=np.float32)) + m)


def _numpy_fallback(logits, x, delta):
    logits = np.asarray(logits, dtype=np.float32)
    x = np.asarray(x, dtype=np.float32)
    delta = np.float32(delta)
    n = logits.shape[1]
    med = np.sort(logits, axis=1)[:, (n - 1) // 2, :]
    std = np.asarray(logits, dtype=np.float32).std(axis=0, ddof=1).astype(np.float32)
    std_med = np.sort(std, axis=0)[(n - 1) // 2, :]
    thresh = med[:, None, :]
    above = (logits >= thresh + FACTOR * std_med) & (logits >= thresh + delta / 2)
    cls = above.astype(np.int32)
    s = cls[:5].sum(axis=0)
    mode = (s >= 3).astype(np.float32)
    c = np.broadcast_to(mode[None], logits.shape).astype(np.float32)
    xs = np.concatenate([np.zeros((x.shape[0], 1), x.dtype), x], axis=1)
    dx = delta * c + xs[:, None, :]
    outs = []
    for i in range(4):
        oth = [j for j in range(4) if j != i]
        m = dx[..., oth].max(axis=-1)
        lse = np.log(np.sum(np.exp(dx[..., oth] - m[..., None]), axis=-1)) + m
        outs.append(dx[..., i] - lse)
    return np.stack(outs, axis=-1).astype(np.float32), c


def _median_lower(v):
    """Exact torch-style lower median of a 1D array."""
    k = (v.shape[0] - 1) // 2
    return np.partition(v, k)[k]


def kernel(logits, x, delta):
    logits = np.ascontiguousarray(np.asarray(logits, dtype=np.float32))
    x = np.asarray(x, dtype=np.float32)
    dval = float(np.asarray(delta))
    if dval != 0.0 or logits.shape != (10, N, 4):
        return _numpy_fallback(logits, x, delta)

    from concourse.bass_utils import run_bass_kernel_spmd

    def _run(nc, in_maps, cores):
        # a wedged accelerator session recovers on a fresh NRT attempt
        import time as _t
        try:
            return run_bass_kernel_spmd(nc, in_maps, core_ids=cores)
        except Exception:
            _t.sleep(5)
            return run_bass_kernel_spmd(nc, in_maps, core_ids=cores)

    cores = list(range(NCORES))

    # exact lower medians of rows 0-4 on host (cheap: ~70ms of np.partition)
    med = np.empty((5, 4), dtype=np.float32)
    for t in range(5):
        for ch in range(4):
            med[t, ch] = _median_lower(np.ascontiguousarray(logits[t, :, ch]))

    lf16 = logits.astype(np.float16)
    in1 = [{"shardf16": _pad_shard16(lf16[:, c * SHARD:(c + 1) * SHARD, :]),
            "medin": med} for c in cores]
    import time as _time
    nc1 = build_fused()
    _t = _time.time()
    r1 = _run(nc1, in1, cores)
    LAST_RUN_TIMES.append(_time.time() - _t)

    qovals = np.concatenate(
        [_trim(r1.results[c]["qo"], SHARD) for c in cores], axis=0
    )  # (N, 4) f16, = q - 8.35
    m3 = np.concatenate(
        [_trim(r1.results[c]["m3"], SHARD) for c in cores], axis=0
    ).astype(np.float32)  # (N, 4)

    qmed = np.empty(4, dtype=np.float32)
    for ch in range(4):
        qv = _median_lower(np.ascontiguousarray(qovals[:, ch]))
        if not (-Q_BRACKET < float(qv) < Q_BRACKET):
            # q median escaped the encodable window (never for sane inputs)
            return _numpy_fallback(logits, x, delta)
        qmed[ch] = np.float32(qv) + Q_OFF
    std_med = np.sqrt(qmed / np.float32(9)).astype(np.float32)

    th = (FACTOR * std_med).astype(np.float32)       # (4,)
    mode = m3 >= th[None, :]                         # (N, 4) bool
    # exact re-check of columns the f16 m3 cannot decide: f16 input
    # quantization (2^-11 * max|x|) + f16 output rounding, plus slack
    maxabs = float(np.max(np.abs(logits[:5])))
    margin = np.float32(max(0.0045, 2.0 ** -11 * maxabs + 8e-4))
    amb_n, amb_ch = np.nonzero(np.abs(m3 - th[None, :]) < margin)
    for n, ch in zip(amb_n, amb_ch):
        t1 = med[:, ch] + np.float32(FACTOR * std_med[ch])
        cnt = int((logits[:5, n, ch] >= t1).sum())
        mode[n, ch] = cnt >= 3
    mode = mode.astype(np.float32)

    # ---- host assembly ----
    xs = np.concatenate([np.zeros((x.shape[0], 1), np.float32), x], axis=1)
    table = np.zeros((10, 4), dtype=np.float32)
    for t in range(10):
        for i in range(4):
            oth = [j for j in range(4) if j != i]
            table[t, i] = xs[t, i] - _logsumexp_f32(xs[t, oth])
    out_full = np.broadcast_to(table[:, None, :], (10, N, 4))
    c_full = np.broadcast_to(mode[None], (10, N, 4))
    return out_full, c_full


# revision 8
# speedup vs baseline: 5.1222x; 1.4097x over previous
"""Trainium2 Bass kernel for nn_DeltaModel (histogram_binning).

Reference semantics (delta == 0, the shipped configuration):
  med[t,ch]   = lower median over N of logits[t,:,ch]   (only rows 0-4 used)
  q[n,ch]     = sumsq - 0.1*sum^2 over the 10 rows      (q/9 = unbiased var)
  std_med[ch] = sqrt(median_N(q[:,ch]) / 9)
  mode[n,ch]  = (#{t<5: logits[t,n,ch] >= med[t,ch] + 1.96*std_med[ch]} >= 3)
              = (median5_t(logits[t,n,ch] - med[t,ch]) >= 1.96*std_med[ch])
  c           = broadcast(mode) over dim 0
  out[t,:,ch] = xs[t,ch] - logsumexp(xs[t,others(ch)])  (constant over N)

Single SPMD launch over 8 NeuronCores (one compile, one transfer): each
core streams its column shard once, producing per-column q (as q-8.35 in
f16) and the median-of-5 of the med-shifted rows 0-4 (m3, f16).  The
transfer link (~60MB/s) dominates, so the shard ships as float16: the
q median only moves ~1e-5 (well under the ~1e-5 std_med budget), and m3's
<=3.4e-3 quantization error is absorbed by an exact host re-check of the
~50 columns that land within `margin` of the threshold.  The host
supplies exact med[t,ch] (np.partition), finishes qmed by partition,
thresholds m3, re-checks the ambiguous columns against the full-precision
logits it already holds, and assembles the broadcast outputs.
"""

import numpy as np

LAST_RUN_TIMES = []  # wall seconds of each device launch (incl. first-call compile)

N = 1_000_000
NCORES = 8
SHARD = N // NCORES            # 125000
PADW_PP = 992                  # per-partition padded columns
SHARD_PAD = 128 * PADW_PP      # 126976
NCHUNK = 2
FACTOR = np.float32(1.96)
Q_OFF = np.float32(8.35)       # chi^2_9 median ~8.34
Q_CLAMP = 0.124                # q-Q_OFF clamped to +-Q_CLAMP before int16 encode
Q_SCALE = 262144.0             # int16 = (q-Q_OFF)*Q_SCALE, grid 3.8e-6
Q_BRACKET = 0.12               # decoded qmed must land strictly inside the clamp
M3_OFF = np.float32(1.886)     # ~F*E[std_med]; m3 ships as (m3-M3_OFF) int8
M3_CLAMP = 0.0635              # m3-M3_OFF clamped to +-M3_CLAMP before encode
M3_SCALE = 2000.0              # int8 = (m3-M3_OFF)*M3_SCALE, grid 5e-4
TH_BRACKET = 0.055             # |th - M3_OFF| must stay below this (else fallback)


def _apply_tile_patch():
    """This walrus build rejects >2 sync waits on the SP Drain emitted at
    TileContext exit ("Too many sync wait commands"); keep one wait on the
    drain and move the rest onto dedicated SP nops before the barrier."""
    import concourse.tile as tile_mod
    from concourse import mybir
    from concourse.vector_clock import ScopedClock

    if getattr(tile_mod.TileContext, "_ant_drain_patched", False):
        return

    def _patched(self, tick_clock, wait_clock):
        nc = self.nc
        drain_inst = nc.sync.drain()
        wait_clock.add_sem_waits(
            drain_inst.ins, ScopedClock({None: tick_clock.global_clock})
        )
        si = drain_inst.ins.sync_info
        if si is not None and si.on_wait is not None and len(si.on_wait) > 1:
            waits = list(si.on_wait)
            drain_inst.ins.sync_info = mybir.SyncInfo(
                on_wait=waits[:1], on_update=list(si.on_update or [])
            )
            for w in waits[1:]:
                nop = nc.sync.nop()
                nop.ins.sync_info = mybir.SyncInfo(on_wait=[w], on_update=[])
        nc.all_engine_barrier()
        assert self.sems is not None
        popped = nc._tile_sem_poison_stack.pop()
        assert popped is self._sem_poison
        nc.clear_and_free_semaphores(list(self.sems.allocated().values()))
        nc.all_engine_barrier()

    tile_mod.TileContext._drain_and_barrier = _patched
    tile_mod.TileContext._ant_drain_patched = True


def _split_sync_waits(nc, maxw=1):
    """This walrus build caps per-instruction sync waits; move excess waits
    onto same-engine NoOps inserted right before the offending instruction."""
    from concourse import mybir

    for f in nc.m.functions:
        for b in f.blocks:
            new_list = []
            changed = False
            for ins in b.instructions:
                si = getattr(ins, "sync_info", None)
                if si is not None and si.on_wait and len(si.on_wait) > maxw:
                    waits = list(si.on_wait)
                    extra, keep = waits[:-maxw], waits[-maxw:]
                    for i in range(0, len(extra), maxw):
                        nop = mybir.InstNoOp(
                            name=f"{ins.name}-wsplit{i}", ins=[], outs=[]
                        )
                        nop.engine = ins.engine
                        nop.sync_info = mybir.SyncInfo(
                            on_wait=extra[i:i + maxw], on_update=[]
                        )
                        new_list.append(nop)
                        changed = True
                    ins.sync_info = mybir.SyncInfo(
                        on_wait=keep, on_update=list(si.on_update or [])
                    )
                new_list.append(ins)
            if changed:
                b.instructions = new_list


def build_fused(padw_pp=PADW_PP, nchunk=NCHUNK, nrows=10, nmed=5,
                split_waits=True):
    """One pass over the f16 shard: q-8.35 (f16) over the 10 rows and m3 =
    median-of-5 of (row_t - med[t]) for rows 0-4 (f16), both per column."""
    import concourse.bass as bass
    import concourse.tile as tile
    from concourse import mybir

    _apply_tile_patch()
    chunk_pp = padw_pp // nchunk
    qw = padw_pp * 4
    F = chunk_pp * 4                   # free elements per chunk (ch-interleaved)
    nc = bass.Bass("TRN2", target_bir_lowering=False, debug=False, num_devices=1)
    shard = nc.dram_tensor("shardf16", [nrows, 128 * padw_pp, 4], mybir.dt.float16,
                           kind="ExternalInput").ap()
    medin = nc.dram_tensor("medin", [nmed, 4], mybir.dt.float32,
                           kind="ExternalInput").ap()
    qo = nc.dram_tensor("qo", [128, qw], mybir.dt.int16,
                        kind="ExternalOutput").ap()
    m3o = nc.dram_tensor("m3", [128, qw], mybir.dt.int8,
                         kind="ExternalOutput").ap()

    with tile.TileContext(nc) as tc:
        with tc.tile_pool(name="stream", bufs=2) as stream, \
             tc.tile_pool(name="scr", bufs=1) as scr, \
             tc.tile_pool(name="small", bufs=1) as small:
            medb = small.tile([128, nmed * 4], mybir.dt.float32)
            nc.sync.dma_start(
                out=medb,
                in_=bass.AP(tensor=medin.tensor, offset=0,
                            ap=[[0, 128], [1, nmed * 4]]),
            )
            mx = mybir.AluOpType.max
            mn = mybir.AluOpType.min
            add = mybir.AluOpType.add
            for it in range(nchunk):
                ld = stream.tile([128, nrows, F], mybir.dt.float16, tag="ld")
                src = bass.AP(
                    tensor=shard.tensor,
                    offset=it * chunk_pp * 4,
                    ap=[[padw_pp * 4, 128], [128 * padw_pp * 4, nrows],
                        [4, chunk_pp], [1, 4]],
                )
                nc.sync.dma_start(out=ld.rearrange("p t (c k) -> p t c k", k=4),
                                  in_=src)
                # rows 0-4 upcast once; reused by both q and m3 phases
                y = [scr.tile([128, F], mybir.dt.float32, tag=f"y{t}", name=f"y{t}")
                     for t in range(nmed)]
                for t in range(nmed):
                    nc.vector.tensor_copy(y[t], ld[:, t, :])
                # ---- q over all 10 rows ----
                sumt = scr.tile([128, F], mybir.dt.float32, tag="sum", name="sum")
                ssq = scr.tile([128, F], mybir.dt.float32, tag="ssq", name="ssq")
                sq = scr.tile([128, F], mybir.dt.float32, tag="sq", name="sq")
                xf = scr.tile([128, F], mybir.dt.float32, tag="xf", name="xf")
                nc.vector.tensor_copy(sumt, y[0])
                nc.scalar.activation(out=ssq, in_=y[0],
                                     func=mybir.ActivationFunctionType.Square)
                for t in range(1, nrows):
                    if t < nmed:
                        xt = y[t]
                    else:
                        nc.vector.tensor_copy(xf, ld[:, t, :])
                        xt = xf
                    nc.scalar.activation(out=sq, in_=xt,
                                         func=mybir.ActivationFunctionType.Square)
                    nc.vector.tensor_tensor(out=sumt, in0=sumt, in1=xt, op=add)
                    nc.vector.tensor_tensor(out=ssq, in0=ssq, in1=sq, op=add)
                nc.scalar.activation(out=sq, in_=sumt,
                                     func=mybir.ActivationFunctionType.Square)
                nc.vector.scalar_tensor_tensor(
                    out=ssq, in0=sq, scalar=-0.1, in1=ssq,
                    op0=mybir.AluOpType.mult, op1=add,
                )
                # (q - Q_OFF) clamped then scaled to an int16 grid of 3.8e-6
                nc.vector.tensor_scalar(out=sq, in0=ssq, scalar1=-float(Q_OFF),
                                        scalar2=Q_CLAMP, op0=add,
                                        op1=mybir.AluOpType.min)
                q16 = scr.tile([128, F], mybir.dt.int16, tag="q16", name="q16")
                nc.vector.tensor_scalar(out=q16, in0=sq, scalar1=-Q_CLAMP,
                                        scalar2=Q_SCALE, op0=mx,
                                        op1=mybir.AluOpType.mult)
                nc.sync.dma_start(out=qo[:, it * F:(it + 1) * F], in_=q16)

                # ---- m3 over med-shifted rows 0-4 (in-place on y) ----
                for t in range(nmed):
                    medv = bass.AP(tensor=medb.tensor, offset=medb.offset + t * 4,
                                   ap=[medb.ap[0], [0, chunk_pp], [1, 4]])
                    nc.vector.scalar_tensor_tensor(
                        out=y[t].rearrange("p (c k) -> p c k", k=4),
                        in0=medv, scalar=-1.0,
                        in1=y[t].rearrange("p (c k) -> p c k", k=4),
                        op0=mybir.AluOpType.mult, op1=add,
                    )
                s1 = scr.tile([128, F], mybir.dt.float32, tag="s1", name="s1")
                s2 = scr.tile([128, F], mybir.dt.float32, tag="s2", name="s2")
                tt = nc.vector.tensor_tensor
                tt(out=s1, in0=y[0], in1=y[1], op=mx)    # s1 = max01
                tt(out=y[0], in0=y[0], in1=y[1], op=mn)  # y0 = min01
                tt(out=s2, in0=y[2], in1=y[3], op=mx)    # s2 = max23
                tt(out=y[2], in0=y[2], in1=y[3], op=mn)  # y2 = min23
                tt(out=y[0], in0=y[0], in1=y[2], op=mx)  # f = max(min01, min23)
                tt(out=s1, in0=s1, in1=s2, op=mn)        # g = min(max01, max23)
                tt(out=s2, in0=y[4], in1=y[0], op=mx)    # v = max(e, f)
                tt(out=y[4], in0=y[4], in1=y[0], op=mn)  # u = min(e, f)
                tt(out=s2, in0=s2, in1=s1, op=mn)        # w = min(v, g)
                tt(out=s2, in0=y[4], in1=s2, op=mx)      # m3 = max(u, w)
                # (m3 - M3_OFF) clamped then scaled to an int8 grid of 5e-4
                nc.vector.tensor_scalar(out=s1, in0=s2, scalar1=-float(M3_OFF),
                                        scalar2=M3_CLAMP, op0=add,
                                        op1=mybir.AluOpType.min)
                m8 = scr.tile([128, F], mybir.dt.int8, tag="m8", name="m8")
                nc.vector.tensor_scalar(out=m8, in0=s1, scalar1=-M3_CLAMP,
                                        scalar2=M3_SCALE, op0=mx,
                                        op1=mybir.AluOpType.mult)
                nc.sync.dma_start(out=m3o[:, it * F:(it + 1) * F], in_=m8)
    if split_waits:
        _split_sync_waits(nc)
    return nc


def _pad_shard16(lf16_shard, padw_pp=PADW_PP):
    """(10, SHARD, 4) f16 -> (10, 128*padw_pp, 4) zero-padded."""
    nrows, w, chn = lf16_shard.shape
    out = np.zeros((nrows, 128 * padw_pp, chn), dtype=np.float16)
    out[:, :w, :] = lf16_shard
    return out


def _trim(arr128, width, padw_pp=PADW_PP):
    """[128, padw_pp*4] core output -> (width, 4)."""
    return arr128.reshape(128 * padw_pp, 4)[:width]


def _logsumexp_f32(v):
    m = np.max(v)
    return np.float32(np.log(np.sum(np.exp(v - m, dtype=np.float32), dtype=np.float32)) + m)


def _numpy_fallback(logits, x, delta):
    logits = np.asarray(logits, dtype=np.float32)
    x = np.asarray(x, dtype=np.float32)
    delta = np.float32(delta)
    n = logits.shape[1]
    med = np.sort(logits, axis=1)[:, (n - 1) // 2, :]
    std = np.asarray(logits, dtype=np.float32).std(axis=0, ddof=1).astype(np.float32)
    std_med = np.sort(std, axis=0)[(n - 1) // 2, :]
    thresh = med[:, None, :]
    above = (logits >= thresh + FACTOR * std_med) & (logits >= thresh + delta / 2)
    cls = above.astype(np.int32)
    s = cls[:5].sum(axis=0)
    mode = (s >= 3).astype(np.float32)
    c = np.broadcast_to(mode[None], logits.shape).astype(np.float32)
    xs = np.concatenate([np.zeros((x.shape[0], 1), x.dtype), x], axis=1)
    dx = delta * c + xs[:, None, :]
    outs = []
    for i in range(4):
        oth = [j for j in range(4) if j != i]
        m = dx[..., oth].max(axis=-1)
        lse = np.log(np.sum(np.exp(dx[..., oth] - m[..., None]), axis=-1)) + m
        outs.append(dx[..., i] - lse)
    return np.stack(outs, axis=-1).astype(np.float32), c


def _median_lower(v):
    """Exact torch-style lower median of a 1D array."""
    k = (v.shape[0] - 1) // 2
    return np.partition(v, k)[k]


def kernel(logits, x, delta):
    logits = np.ascontiguousarray(np.asarray(logits, dtype=np.float32))
    x = np.asarray(x, dtype=np.float32)
    dval = float(np.asarray(delta))
    if dval != 0.0 or logits.shape != (10, N, 4):
        return _numpy_fallback(logits, x, delta)

    from concourse.bass_utils import run_bass_kernel_spmd

    def _run(nc, in_maps, cores):
        # a wedged accelerator session recovers on a fresh NRT attempt
        import time as _t
        try:
            return run_bass_kernel_spmd(nc, in_maps, core_ids=cores)
        except Exception:
            _t.sleep(5)
            return run_bass_kernel_spmd(nc, in_maps, core_ids=cores)

    cores = list(range(NCORES))

    # exact lower medians of rows 0-4 on host (cheap: ~70ms of np.partition)
    med = np.empty((5, 4), dtype=np.float32)
    for t in range(5):
        for ch in range(4):
            med[t, ch] = _median_lower(np.ascontiguousarray(logits[t, :, ch]))

    lf16 = logits.astype(np.float16)
    in1 = [{"shardf16": _pad_shard16(lf16[:, c * SHARD:(c + 1) * SHARD, :]),
            "medin": med} for c in cores]
    import time as _time
    nc1 = build_fused()
    _t = _time.time()
    r1 = _run(nc1, in1, cores)
    LAST_RUN_TIMES.append(_time.time() - _t)

    qovals = np.concatenate(
        [_trim(r1.results[c]["qo"], SHARD) for c in cores], axis=0
    )  # (N, 4) int16, = (q - Q_OFF) * Q_SCALE
    m3 = np.concatenate(
        [_trim(r1.results[c]["m3"], SHARD) for c in cores], axis=0
    ).astype(np.float32) * np.float32(1.0 / M3_SCALE) + M3_OFF  # (N, 4)

    qmed = np.empty(4, dtype=np.float32)
    for ch in range(4):
        qv = float(_median_lower(np.ascontiguousarray(qovals[:, ch]))) / Q_SCALE
        if not (-Q_BRACKET < qv < Q_BRACKET):
            # q median escaped the encodable window (never for sane inputs)
            return _numpy_fallback(logits, x, delta)
        qmed[ch] = np.float32(qv) + Q_OFF
    std_med = np.sqrt(qmed / np.float32(9)).astype(np.float32)

    th = (FACTOR * std_med).astype(np.float32)       # (4,)
    if bool(np.any(np.abs(th - M3_OFF) > TH_BRACKET)):
        # threshold escaped the m3 encodable window (never for sane inputs)
        return _numpy_fallback(logits, x, delta)
    mode = m3 >= th[None, :]                         # (N, 4) bool
    # exact re-check of columns the encoded m3 cannot decide: f16 input
    # quantization (2^-11 * max|x|) + subtract rounding + int8 grid + slack
    maxabs = float(np.max(np.abs(logits[:5])))
    margin = np.float32(max(0.005, 2.0 ** -11 * maxabs + 1.5e-3))
    amb_n, amb_ch = np.nonzero(np.abs(m3 - th[None, :]) < margin)
    for n, ch in zip(amb_n, amb_ch):
        t1 = med[:, ch] + np.float32(FACTOR * std_med[ch])
        cnt = int((logits[:5, n, ch] >= t1).sum())
        mode[n, ch] = cnt >= 3
    mode = mode.astype(np.float32)

    # ---- host assembly ----
    xs = np.concatenate([np.zeros((x.shape[0], 1), np.float32), x], axis=1)
    table = np.zeros((10, 4), dtype=np.float32)
    for t in range(10):
        for i in range(4):
            oth = [j for j in range(4) if j != i]
            table[t, i] = xs[t, i] - _logsumexp_f32(xs[t, oth])
    out_full = np.broadcast_to(table[:, None, :], (10, N, 4))
    c_full = np.broadcast_to(mode[None], (10, N, 4))
    return out_full, c_full
